# revision 1
# baseline (speedup 1.0000x reference)
"""Trainium2 Bass kernel for nn_DDA_PU_loss.

loss = sum((A-B)[pos]^2) * (1-alpha)/2 + sum((A-B)[neg]^2) * alpha/2
with A = drug_virus_reconstruct [8192, 16384], B = drug_virus [8192, 16384],
pos/neg given as 524288 / 2097152 random (x, y) int64 index pairs.
(drug_virus_mask is unused by the reference.)

Strategy (data-parallel row-shard):
  * Row-shard A, B into 8 blocks of 1024 rows (one per NeuronCore).
  * Host-side index prep (index-only, no value compute): bucket the index
    pairs by row-block and bincount them into per-cell multiplicities;
    build a sparse sqrt-weight matrix
        m = sqrt((wp * count_pos + wn * count_neg) / scale),
    wp = (1-alpha)/2, wn = alpha/2, scale = dominant class weight — ~2%
    nonzero, streamed as dithered fp16 (per-cell choice between the two
    adjacent fp16 values of sqrt(w) so E[m^2] == w exactly; the rounding
    averages out to ~1e-6 relative over the ~300k nonzero cells/core).
  * Device per core: stream A, B, m tiles through SBUF; DVE computes
    acc += ((A - B) * m)^2, which equals diff^2 * weight per cell;
    final free-axis reduce -> [128, 1] partials.
  * Host: loss = scale * sum of the 8 x 128 partials.

Measured ~0.51 ms device time per core (8 cores in parallel), at ~90% of
the 160 MiB/core / ~358 GB/s HBM streaming roofline.  Per-element gathers
were measured and rejected: SWDGE indirect DMA gathers one index per
partition (128 single elements per ~1.2 us instruction) and gpsimd
ap_gather costs ~43 ns/index — both >= 5x slower than dense streaming at
this 2% index density.  The gathered-sum formulation is exactly equivalent
because the loss is a multiplicity-weighted sum of squared diffs over
cells.
"""

import numpy as np

N_DRUGS = 8192
N_VIRUS = 16384
N_CORES = 8
ROWS_PER_CORE = N_DRUGS // N_CORES  # 1024

FULL_CFG = dict(
    n_cores=N_CORES,
    rows_per_core=ROWS_PER_CORE,
    n_virus=N_VIRUS,
    tile_f=4096,   # free-dim tile size -> [128, 4096] f32 = 2 MiB per stream
    mask_f16=True,  # stream the sqrt-weight mask as fp16 (halves its traffic)
)

TRACE = False
LAST_RESULTS = None

_BUILD_CACHE = {}


def build_nc(cfg):
    import concourse.tile as tile
    from concourse import bacc, mybir

    R = cfg["rows_per_core"]
    V = cfg["n_virus"]
    TF = cfg["tile_f"]
    n_rt = R // 128
    n_ft = V // TF

    nc = bacc.Bacc(
        "TRN2",
        target_bir_lowering=False,
        debug=False,
        num_devices=cfg["n_cores"],
    )
    mdt = mybir.dt.float16 if cfg.get("mask_f16") else mybir.dt.float32
    a = nc.dram_tensor("a", [R, V], mybir.dt.float32, kind="ExternalInput").ap()
    b = nc.dram_tensor("b", [R, V], mybir.dt.float32, kind="ExternalInput").ap()
    m = nc.dram_tensor("m", [R, V], mdt, kind="ExternalInput").ap()
    partials = nc.dram_tensor(
        "partials", [128, 1], mybir.dt.float32, kind="ExternalOutput"
    ).ap()

    with tile.TileContext(nc) as tc:
        with tc.tile_pool(name="str", bufs=cfg.get("bufs", 4)) as spool, \
             tc.tile_pool(name="small", bufs=1) as small_pool:

            acc = small_pool.tile([128, TF], mybir.dt.float32)
            nc.vector.memset(acc[:], 0.0)

            for _rep in range(cfg.get("repeat", 1)):
              for rt in range(n_rt):
                for ft in range(n_ft):
                    rsl = slice(rt * 128, rt * 128 + 128)
                    fsl = slice(ft * TF, (ft + 1) * TF)
                    at = spool.tile([128, TF], mybir.dt.float32, tag="at")
                    nc.sync.dma_start(out=at[:], in_=a[rsl, fsl])
                    bt = spool.tile([128, TF], mybir.dt.float32, tag="bt")
                    nc.scalar.dma_start(out=bt[:], in_=b[rsl, fsl])
                    mt = spool.tile([128, TF], mdt, tag="mt")
                    if cfg.get("m_hwdge"):
                        meng = nc.sync if (rt * n_ft + ft) % 2 == 0 else nc.scalar
                    else:
                        meng = nc.gpsimd
                    meng.dma_start(out=mt[:], in_=m[rsl, fsl])
                    # d = a - b (in-place into at; optionally on gpsimd to
                    # offload DVE — independent across tiles, no serial chain)
                    seng = nc.gpsimd if cfg.get("sub_gpsimd") else nc.vector
                    seng.tensor_tensor(
                        out=at[:], in0=at[:], in1=bt[:],
                        op=mybir.AluOpType.subtract,
                    )
                    # t = d * m (in-place into at)
                    nc.vector.tensor_tensor(
                        out=at[:], in0=at[:], in1=mt[:],
                        op=mybir.AluOpType.mult,
                    )
                    # sq = t * t (into bt)
                    nc.vector.tensor_tensor(
                        out=bt[:], in0=at[:], in1=at[:],
                        op=mybir.AluOpType.mult,
                    )
                    # acc += sq (optionally on gpsimd to offload DVE)
                    aeng = nc.gpsimd if cfg.get("acc_gpsimd") else nc.vector
                    aeng.tensor_tensor(
                        out=acc[:], in0=acc[:], in1=bt[:],
                        op=mybir.AluOpType.add,
                    )

            red = small_pool.tile([128, 1], mybir.dt.float32)
            nc.vector.tensor_reduce(
                out=red[:], in_=acc[:],
                axis=mybir.AxisListType.X, op=mybir.AluOpType.add,
            )
            nc.sync.dma_start(out=partials[:, :], in_=red[:])

    nc.compile()
    return nc


def build_masks(pos_x, pos_y, neg_x, neg_y, alpha, cfg):
    """Index-only host prep: per-core sqrt-weight matrices [R, V].

    Returns (masks, scale): the device computes sum(d^2 * m^2); the final
    loss is scale * sum(partials).  Weights are rescaled by the dominant
    class weight so that the vast majority of nonzero mask cells are
    exactly 1.0 — exactly representable in fp16 — making the fp16 mask
    essentially lossless for the dominant class.
    """
    R = cfg["rows_per_core"]
    V = cfg["n_virus"]
    n_cores = cfg["n_cores"]
    wp = (1.0 - float(alpha)) / 2.0
    wn = float(alpha) / 2.0
    px = np.asarray(pos_x).astype(np.int64, copy=False)
    py = np.asarray(pos_y).astype(np.int64, copy=False)
    nx = np.asarray(neg_x).astype(np.int64, copy=False)
    ny = np.asarray(neg_y).astype(np.int64, copy=False)
    # dominant weight-mass class defines the scale (mask value 1.0)
    mass_p = wp * len(px)
    mass_n = wn * len(nx)
    scale = wn if mass_n >= mass_p else wp
    if scale == 0.0:
        scale = max(wp, wn, 1e-30)
    pflat = px * V + py
    nflat = nx * V + ny
    pcore = px // R
    ncore = nx // R
    shard = R * V
    masks = []
    for c in range(n_cores):
        pl = pflat[pcore == c] - c * shard
        nl = nflat[ncore == c] - c * shard
        cp = np.bincount(pl, minlength=shard)
        cn = np.bincount(nl, minlength=shard)
        w = (wp / scale) * cp.astype(np.float32) + (wn / scale) * cn.astype(
            np.float32
        )
        if cfg.get("mask_f16"):
            # dithered fp16 sqrt-weights: pick between the two adjacent fp16
            # values of sqrt(w) per cell so that E[m_f16^2] == w exactly;
            # the per-cell rounding becomes zero-mean noise that averages
            # out over the ~300k nonzero cells (~1e-6 relative).
            nz = np.flatnonzero(w)
            wv = w[nz]
            m0 = np.sqrt(wv).astype(np.float16)
            w0 = m0.astype(np.float32) ** 2
            toward = np.where(w0 < wv, np.float16(np.inf), np.float16(0.0))
            malt = np.nextafter(m0, toward)
            walt = malt.astype(np.float32) ** 2
            denom = w0 - walt
            q = np.where(denom != 0, (wv - walt) / np.where(denom == 0, 1, denom), 1.0)
            nzu = nz.astype(np.uint64)
            u = (
                ((nzu * np.uint64(2654435761)) & np.uint64(0xFFFFFFFF)) >> np.uint64(16)
            ).astype(np.float64) / 65536.0
            mv = np.where(u < q, m0, malt)
            mf = np.zeros(shard, dtype=np.float16)
            mf[nz] = mv
            masks.append(mf.reshape(R, V))
        else:
            np.sqrt(w, out=w)
            masks.append(w.reshape(R, V))
    return masks, scale


def run_cores(in_maps, cfg):
    global LAST_RESULTS
    from concourse.bass_utils import run_bass_kernel_spmd
    from concourse.bass_interp import get_hw_module

    key = tuple(sorted(cfg.items()))
    if key not in _BUILD_CACHE:
        _BUILD_CACHE[key] = build_nc(cfg)
    nc = _BUILD_CACHE[key]

    old_m = nc.m
    nc.m = get_hw_module(nc.m)
    try:
        res = run_bass_kernel_spmd(
            nc,
            in_maps,
            core_ids=list(range(len(in_maps))),
            trace=TRACE,
        )
    finally:
        nc.m = old_m
    LAST_RESULTS = res
    return [r["partials"] for r in res.results]


def kernel(
    drug_virus_reconstruct,
    drug_virus,
    drug_virus_mask,
    pos_x_index,
    pos_y_index,
    neg_x_index,
    neg_y_index,
    alpha,
):
    cfg = FULL_CFG
    A = np.ascontiguousarray(np.asarray(drug_virus_reconstruct, dtype=np.float32))
    B = np.ascontiguousarray(np.asarray(drug_virus, dtype=np.float32))
    R = cfg["rows_per_core"]

    masks, scale = build_masks(
        pos_x_index, pos_y_index, neg_x_index, neg_y_index, alpha, cfg
    )

    in_maps = [
        {
            "a": A[c * R: (c + 1) * R],
            "b": B[c * R: (c + 1) * R],
            "m": masks[c],
        }
        for c in range(cfg["n_cores"])
    ]

    partials = run_cores(in_maps, cfg)
    loss = scale * float(
        np.sum([np.sum(p, dtype=np.float64) for p in partials], dtype=np.float64)
    )
    return np.float32(loss)



# revision 3
# speedup vs baseline: 1.3406x; 1.3406x over previous
"""Trainium2 Bass kernel for nn_DDA_PU_loss.

loss = sum((A-B)[pos]^2) * (1-alpha)/2 + sum((A-B)[neg]^2) * alpha/2
with A = drug_virus_reconstruct [8192, 16384], B = drug_virus [8192, 16384],
pos/neg given as 524288 / 2097152 random (x, y) int64 index pairs.
(drug_virus_mask is unused by the reference.)

Strategy (data-parallel row-shard):
  * Row-shard A, B into 8 blocks of 1024 rows (one per NeuronCore).
  * Host-side index prep (index-only, no value compute): bucket the index
    pairs by row-block and bincount them into per-cell multiplicities;
    build a sparse sqrt-weight matrix
        m = sqrt((wp * count_pos + wn * count_neg) / scale),
    wp = (1-alpha)/2, wn = alpha/2, scale = dominant class weight, streamed
    as a dithered low-precision mask (per-cell choice between the two
    adjacent representable values of sqrt(w) so E[m^2] == w exactly).
  * Device per core, per [128, TF] tile:
      DVE:  d = a - b            (f32 in, f16 out)
      DVE:  u = d * m            (f16, 2x mode when mask is f16)
      Act:  square(u) with accum_out -> per-tile [128,1] partial column
    Final: reduce the [128, n_tiles] partial columns -> [128, 1] -> HBM.
  * Host: loss = scale * sum of the 8 x 128 partials.

The pass is HBM-streaming bound: per core it reads A (64 MiB f32),
B (64 MiB f32) and the mask (16 MiB fp8 / 32 MiB f16).  Engine work per
[128, 2048] tile is DVE ~3.2-4.3 us, Act ~1.7 us against ~13-15 us of DMA,
so DVE/Act never gate.  Per-element gathers were measured and rejected:
SWDGE indirect DMA gathers 128 single elements per ~1.2 us instruction and
gpsimd ap_gather costs ~43 ns/index — both >= 5x slower than dense
streaming at this ~2% index density.  The gathered-sum formulation is
exactly equivalent because the loss is a multiplicity-weighted sum of
squared diffs over cells.
"""

import numpy as np

N_DRUGS = 8192
N_VIRUS = 16384
N_CORES = 8
ROWS_PER_CORE = N_DRUGS // N_CORES  # 1024

FULL_CFG = dict(
    n_cores=N_CORES,
    rows_per_core=ROWS_PER_CORE,
    n_virus=N_VIRUS,
    tile_f=2048,        # free-dim tile size
    bufs=6,             # tile-pool rotation depth
    mask_dtype="fp8e4",  # "f16" | "fp8e4" | "fp8e5"
    b_queue="scalar",   # engine queue issuing B's DMA (SP/Act/gpsimd only)
    m_queue="gpsimd",   # engine queue issuing the mask DMA
)

TRACE = False
LAST_RESULTS = None

_BUILD_CACHE = {}


def _mask_mybir_dt(cfg):
    from concourse import mybir

    return {
        "f16": mybir.dt.float16,
        "fp8e4": mybir.dt.float8e4,
        "fp8e5": mybir.dt.float8e5,
    }[cfg.get("mask_dtype", "f16")]


def _mask_np_dt(cfg):
    from concourse import mybir

    return mybir.dt.np(_mask_mybir_dt(cfg))


def build_nc(cfg):
    import concourse.tile as tile
    from concourse import bacc, mybir

    R = cfg["rows_per_core"]
    V = cfg["n_virus"]
    TF = cfg["tile_f"]
    n_rt = R // 128
    n_ft = V // TF
    n_tiles = n_rt * n_ft

    nc = bacc.Bacc(
        "TRN2",
        target_bir_lowering=False,
        debug=False,
        num_devices=cfg["n_cores"],
    )
    mdt = _mask_mybir_dt(cfg)
    a = nc.dram_tensor("a", [R, V], mybir.dt.float32, kind="ExternalInput").ap()
    b = nc.dram_tensor("b", [R, V], mybir.dt.float32, kind="ExternalInput").ap()
    m = nc.dram_tensor("m", [R, V], mdt, kind="ExternalInput").ap()
    partials = nc.dram_tensor(
        "partials", [128, 1], mybir.dt.float32, kind="ExternalOutput"
    ).ap()

    dma_only = cfg.get("dma_only", False)
    no_mask = cfg.get("no_mask", False)

    with tile.TileContext(nc) as tc:
        with tc.tile_pool(name="str", bufs=cfg.get("bufs", 6)) as spool, \
             tc.tile_pool(name="small", bufs=1) as small_pool:

            acc = small_pool.tile([128, n_tiles], mybir.dt.float32)
            nc.vector.memset(acc[:], 0.0)

            beng = getattr(nc, cfg.get("b_queue", "tensor"))
            meng = getattr(nc, cfg.get("m_queue", "gpsimd"))

            for _rep in range(cfg.get("repeat", 1)):
              for rt in range(n_rt):
                for ft in range(n_ft):
                    idx = rt * n_ft + ft
                    rsl = slice(rt * 128, rt * 128 + 128)
                    fsl = slice(ft * TF, (ft + 1) * TF)
                    at = spool.tile([128, TF], mybir.dt.float32, tag="at")
                    nc.sync.dma_start(out=at[:], in_=a[rsl, fsl])
                    bt = spool.tile([128, TF], mybir.dt.float32, tag="bt")
                    beng.dma_start(out=bt[:], in_=b[rsl, fsl])
                    if not no_mask:
                        mt = spool.tile([128, TF], mdt, tag="mt")
                        meng.dma_start(out=mt[:], in_=m[rsl, fsl])
                    if dma_only:
                        continue
                    ut = spool.tile([128, TF], mybir.dt.float16, tag="ut")
                    # d = a - b (f32 -> f16)
                    nc.vector.tensor_tensor(
                        out=ut[:], in0=at[:], in1=bt[:],
                        op=mybir.AluOpType.subtract,
                    )
                    # u = d * m
                    if not no_mask:
                        nc.vector.tensor_tensor(
                            out=ut[:], in0=ut[:], in1=mt[:],
                            op=mybir.AluOpType.mult,
                        )
                    # acc[:, idx] = sum_free(u^2) on the Act engine
                    nc.scalar.activation(
                        out=ut[:], in_=ut[:],
                        func=mybir.ActivationFunctionType.Square,
                        accum_out=acc[:, idx:idx + 1],
                    )

            red = small_pool.tile([128, 1], mybir.dt.float32)
            nc.vector.tensor_reduce(
                out=red[:], in_=acc[:],
                axis=mybir.AxisListType.X, op=mybir.AluOpType.add,
            )
            nc.sync.dma_start(out=partials[:, :], in_=red[:])

    nc.compile()
    return nc


def _dither_sqrt(w_nz, np_dtype):
    """Per-cell choice between the two adjacent `np_dtype` values of
    sqrt(w) such that E[m^2] == w exactly (deterministic hash dither)."""
    m0 = np.sqrt(w_nz).astype(np_dtype)
    w0 = m0.astype(np.float32) ** 2
    up = w0 < w_nz
    if np_dtype == np.float16:
        toward = np.where(up, np.float16(np.inf), np.float16(0.0))
        malt = np.nextafter(m0, toward)
    else:
        # fp8 (ml_dtypes): adjacent representable value via byte bump;
        # monotonic for positive finite values.
        u = m0.view(np.uint8)
        ualt = (u + np.where(up, 1, -1).astype(np.uint8)).astype(np.uint8)
        ualt[u == 0] = 1  # nextafter(0) = smallest subnormal
        malt = ualt.view(np_dtype)
    walt = malt.astype(np.float32) ** 2
    denom = w0 - walt
    q = np.where(denom != 0, (w_nz - walt) / np.where(denom == 0, 1, denom), 1.0)
    return m0, malt, np.clip(q, 0.0, 1.0)


def build_masks(pos_x, pos_y, neg_x, neg_y, alpha, cfg):
    """Index-only host prep: per-core sqrt-weight matrices [R, V].

    Returns (masks, scale): the device computes sum(d^2 * m^2); the final
    loss is scale * sum(partials).  Weights are rescaled by the dominant
    class weight so that the vast majority of nonzero mask cells are
    exactly 1.0 — exactly representable in f16/fp8 — making the dithered
    low-precision mask essentially lossless for the dominant class.
    """
    R = cfg["rows_per_core"]
    V = cfg["n_virus"]
    n_cores = cfg["n_cores"]
    np_mdt = _mask_np_dt(cfg)
    wp = (1.0 - float(alpha)) / 2.0
    wn = float(alpha) / 2.0
    px = np.asarray(pos_x).astype(np.int64, copy=False)
    py = np.asarray(pos_y).astype(np.int64, copy=False)
    nx = np.asarray(neg_x).astype(np.int64, copy=False)
    ny = np.asarray(neg_y).astype(np.int64, copy=False)
    # dominant weight-mass class defines the scale (mask value 1.0)
    mass_p = wp * len(px)
    mass_n = wn * len(nx)
    scale = wn if mass_n >= mass_p else wp
    if scale == 0.0:
        scale = max(wp, wn, 1e-30)
    pflat = px * V + py
    nflat = nx * V + ny
    pcore = px // R
    ncore = nx // R
    shard = R * V
    masks = []
    for c in range(n_cores):
        pl = pflat[pcore == c] - c * shard
        nl = nflat[ncore == c] - c * shard
        cp = np.bincount(pl, minlength=shard)
        cn = np.bincount(nl, minlength=shard)
        w = (wp / scale) * cp.astype(np.float32) + (wn / scale) * cn.astype(
            np.float32
        )
        nz = np.flatnonzero(w)
        wv = w[nz]
        m0, malt, q = _dither_sqrt(wv, np_mdt)
        nzu = nz.astype(np.uint64)
        u = (
            ((nzu * np.uint64(2654435761)) & np.uint64(0xFFFFFFFF)) >> np.uint64(16)
        ).astype(np.float64) / 65536.0
        mv = np.where(u < q, m0, malt)
        mf = np.zeros(shard, dtype=np_mdt)
        mf[nz] = mv
        masks.append(mf.reshape(R, V))
    return masks, scale


def run_cores(in_maps, cfg):
    global LAST_RESULTS
    from concourse.bass_utils import run_bass_kernel_spmd
    from concourse.bass_interp import get_hw_module

    key = tuple(sorted(cfg.items()))
    if key not in _BUILD_CACHE:
        _BUILD_CACHE[key] = build_nc(cfg)
    nc = _BUILD_CACHE[key]

    old_m = nc.m
    nc.m = get_hw_module(nc.m)
    try:
        res = run_bass_kernel_spmd(
            nc,
            in_maps,
            core_ids=list(range(len(in_maps))),
            trace=TRACE,
        )
    finally:
        nc.m = old_m
    LAST_RESULTS = res
    return [r["partials"] for r in res.results]


def kernel(
    drug_virus_reconstruct,
    drug_virus,
    drug_virus_mask,
    pos_x_index,
    pos_y_index,
    neg_x_index,
    neg_y_index,
    alpha,
):
    cfg = FULL_CFG
    A = np.ascontiguousarray(np.asarray(drug_virus_reconstruct, dtype=np.float32))
    B = np.ascontiguousarray(np.asarray(drug_virus, dtype=np.float32))
    R = cfg["rows_per_core"]

    masks, scale = build_masks(
        pos_x_index, pos_y_index, neg_x_index, neg_y_index, alpha, cfg
    )

    in_maps = [
        {
            "a": A[c * R: (c + 1) * R],
            "b": B[c * R: (c + 1) * R],
            "m": masks[c],
        }
        for c in range(cfg["n_cores"])
    ]

    partials = run_cores(in_maps, cfg)
    loss = scale * float(
        np.sum([np.sum(p, dtype=np.float64) for p in partials], dtype=np.float64)
    )
    return np.float32(loss)


# revision 14
# speedup vs baseline: 1.3505x; 1.0074x over previous
"""Trainium2 Bass kernel for nn_DDA_PU_loss.

loss = sum((A-B)[pos]^2) * (1-alpha)/2 + sum((A-B)[neg]^2) * alpha/2
with A = drug_virus_reconstruct [8192, 16384], B = drug_virus [8192, 16384],
pos/neg given as 524288 / 2097152 random (x, y) int64 index pairs.
(drug_virus_mask is unused by the reference.)

Strategy (data-parallel row-shard):
  * Row-shard A, B into 8 blocks of 1024 rows (one per NeuronCore).
  * Host-side index prep (index-only, no value compute): bucket the index
    pairs by row-block and bincount them into per-cell multiplicities;
    build a sparse sqrt-weight matrix
        m = sqrt((wp * count_pos + wn * count_neg) / scale),
    wp = (1-alpha)/2, wn = alpha/2, scale = dominant class weight, streamed
    as a dithered low-precision mask (per-cell choice between the two
    adjacent representable values of sqrt(w) so E[m^2] == w exactly).
  * Device per core, per [128, TF] tile:
      DVE:  d = a - b            (f32 in, f16 out)
      DVE:  u = d * m            (f16, 2x mode when mask is f16)
      Act:  square(u) with accum_out -> per-tile [128,1] partial column
    Final: reduce the [128, n_tiles] partial columns -> [128, 1] -> HBM.
  * Host: loss = scale * sum of the 8 x 128 partials.

The pass is HBM-streaming bound: per core it reads A (64 MiB f32),
B (64 MiB f32) and the mask (16 MiB fp8e4, dithered).  Measured ~433 us
per pass = 144 MiB at ~349 GB/s/core, ~97% of the 360 GB/s per-core DMA
roofline (22.5 B/ns x 16 DMA engines).  Engine work per [128, 2048] tile
(DVE sub 2.1 us + mult 2.1 us, Act square+accum 1.7 us) sits well under
the ~6.3 us of tile DMA, so only bytes matter.  DMA streams keep fixed
queue affinity (a->SP, b->Act, m->gpsimd/SWDGE); rotating queues or
host-pretiling tiles to enlarge descriptors measured neutral-to-worse.

Measured and rejected:
  * f16 mask (32 MiB): +90 us (stream is pure extra bytes).
  * Bit-plane masks (2 bit/cell, 4 MiB): saves ~35 us of DMA but the
    on-device expansion (uint8 bit-extract + gating) adds >=6 us/tile of
    1x-rate DVE work against ~2 us of slack — strictly worse.
  * Per-element gathers: SWDGE indirect DMA moves 128 single elements
    per ~1.2 us instruction and gpsimd ap_gather costs ~43 ns/index —
    both >= 5x slower than dense streaming at this ~2% index density.
The gathered-sum formulation is exactly equivalent because the loss is a
multiplicity-weighted sum of squared diffs over cells.
"""

import numpy as np

N_DRUGS = 8192
N_VIRUS = 16384
N_CORES = 8
ROWS_PER_CORE = N_DRUGS // N_CORES  # 1024

FULL_CFG = dict(
    n_cores=N_CORES,
    rows_per_core=ROWS_PER_CORE,
    n_virus=N_VIRUS,
    tile_f=2048,        # free-dim tile size
    bufs=6,             # tile-pool rotation depth
    mask_dtype="fp8e4",  # "f16" | "fp8e4" | "fp8e5"
    b_queue="scalar",   # engine queue issuing B's DMA (SP/Act/gpsimd only)
    m_queue="gpsimd",   # engine queue issuing the mask DMA
)

TRACE = False
LAST_RESULTS = None

_BUILD_CACHE = {}


def _mask_mybir_dt(cfg):
    from concourse import mybir

    return {
        "f16": mybir.dt.float16,
        "fp8e4": mybir.dt.float8e4,
        "fp8e5": mybir.dt.float8e5,
    }[cfg.get("mask_dtype", "f16")]


def _mask_np_dt(cfg):
    from concourse import mybir

    return mybir.dt.np(_mask_mybir_dt(cfg))


def build_nc(cfg):
    import concourse.tile as tile
    from concourse import bacc, mybir

    R = cfg["rows_per_core"]
    V = cfg["n_virus"]
    TF = cfg["tile_f"]
    n_rt = R // 128
    n_ft = V // TF
    n_tiles = n_rt * n_ft

    nc = bacc.Bacc(
        "TRN2",
        target_bir_lowering=False,
        debug=False,
        num_devices=cfg["n_cores"],
    )
    mdt = _mask_mybir_dt(cfg)
    pretiled = cfg.get("pretiled", False)
    m_pretiled = pretiled or cfg.get("m_pretiled", False)
    dram_shape = [n_tiles * 128, TF] if pretiled else [R, V]
    m_shape = [n_tiles * 128, TF] if m_pretiled else [R, V]
    a = nc.dram_tensor("a", dram_shape, mybir.dt.float32, kind="ExternalInput").ap()
    b = nc.dram_tensor("b", dram_shape, mybir.dt.float32, kind="ExternalInput").ap()
    m = nc.dram_tensor("m", m_shape, mdt, kind="ExternalInput").ap()
    partials = nc.dram_tensor(
        "partials", [128, 1], mybir.dt.float32, kind="ExternalOutput"
    ).ap()

    dma_only = cfg.get("dma_only", False)
    no_mask = cfg.get("no_mask", False)

    with tile.TileContext(nc) as tc:
        with tc.tile_pool(name="str", bufs=cfg.get("bufs", 6)) as spool, \
             tc.tile_pool(name="small", bufs=1) as small_pool:

            acc = small_pool.tile([128, n_tiles], mybir.dt.float32)
            nc.vector.memset(acc[:], 0.0)

            beng = getattr(nc, cfg.get("b_queue", "scalar"))
            meng = getattr(nc, cfg.get("m_queue", "gpsimd"))
            qmode = cfg.get("queue_mode", "fixed")
            q3 = [nc.sync, nc.scalar, nc.gpsimd]

            for _rep in range(cfg.get("repeat", 1)):
              for rt in range(n_rt):
                for ft in range(n_ft):
                    idx = rt * n_ft + ft
                    if pretiled:
                        rsl = slice(idx * 128, idx * 128 + 128)
                        fsl = slice(0, TF)
                    else:
                        rsl = slice(rt * 128, rt * 128 + 128)
                        fsl = slice(ft * TF, (ft + 1) * TF)
                    if qmode == "rotate":
                        aq = q3[idx % 3]
                        bq = q3[(idx + 1) % 3]
                        mq = q3[(idx + 2) % 3]
                    elif qmode == "swap2":
                        aq = q3[idx % 2]
                        bq = q3[(idx + 1) % 2]
                        mq = nc.gpsimd
                    else:
                        aq, bq, mq = nc.sync, beng, meng
                    at = spool.tile([128, TF], mybir.dt.float32, tag="at")
                    aq.dma_start(out=at[:], in_=a[rsl, fsl])
                    bt = spool.tile([128, TF], mybir.dt.float32, tag="bt")
                    bq.dma_start(out=bt[:], in_=b[rsl, fsl])
                    if not no_mask:
                        mt = spool.tile([128, TF], mdt, tag="mt")
                        if m_pretiled:
                            mq.dma_start(
                                out=mt[:],
                                in_=m[idx * 128:(idx + 1) * 128, :],
                            )
                        else:
                            mq.dma_start(out=mt[:], in_=m[rsl, fsl])
                    if dma_only:
                        continue
                    ut = spool.tile([128, TF], mybir.dt.float16, tag="ut")
                    # d = a - b (f32 -> f16)
                    nc.vector.tensor_tensor(
                        out=ut[:], in0=at[:], in1=bt[:],
                        op=mybir.AluOpType.subtract,
                    )
                    # u = d * m
                    if not no_mask:
                        nc.vector.tensor_tensor(
                            out=ut[:], in0=ut[:], in1=mt[:],
                            op=mybir.AluOpType.mult,
                        )
                    # acc[:, idx] = sum_free(u^2) on the Act engine
                    nc.scalar.activation(
                        out=ut[:], in_=ut[:],
                        func=mybir.ActivationFunctionType.Square,
                        accum_out=acc[:, idx:idx + 1],
                    )

            red = small_pool.tile([128, 1], mybir.dt.float32)
            nc.vector.tensor_reduce(
                out=red[:], in_=acc[:],
                axis=mybir.AxisListType.X, op=mybir.AluOpType.add,
            )
            nc.sync.dma_start(out=partials[:, :], in_=red[:])

    nc.compile()
    return nc


def pretile(arr, cfg):
    """Rearrange a per-core [R, V] array into tile-contiguous layout
    [n_tiles*128, TF] matching build_nc's pretiled slicing (pure layout
    permutation, done host-side at upload)."""
    R = cfg["rows_per_core"]
    V = cfg["n_virus"]
    TF = cfg["tile_f"]
    n_rt = R // 128
    n_ft = V // TF
    return np.ascontiguousarray(
        arr.reshape(n_rt, 128, n_ft, TF)
        .transpose(0, 2, 1, 3)
        .reshape(n_rt * n_ft * 128, TF)
    )


def _dither_sqrt(w_nz, np_dtype):
    """Per-cell choice between the two adjacent `np_dtype` values of
    sqrt(w) such that E[m^2] == w exactly (deterministic hash dither)."""
    m0 = np.sqrt(w_nz).astype(np_dtype)
    w0 = m0.astype(np.float32) ** 2
    up = w0 < w_nz
    if np_dtype == np.float16:
        toward = np.where(up, np.float16(np.inf), np.float16(0.0))
        malt = np.nextafter(m0, toward)
    else:
        # fp8 (ml_dtypes): adjacent representable value via byte bump;
        # monotonic for positive finite values.
        u = m0.view(np.uint8)
        ualt = (u + np.where(up, 1, -1).astype(np.uint8)).astype(np.uint8)
        ualt[u == 0] = 1  # nextafter(0) = smallest subnormal
        malt = ualt.view(np_dtype)
    walt = malt.astype(np.float32) ** 2
    denom = w0 - walt
    q = np.where(denom != 0, (w_nz - walt) / np.where(denom == 0, 1, denom), 1.0)
    return m0, malt, np.clip(q, 0.0, 1.0)


def build_masks(pos_x, pos_y, neg_x, neg_y, alpha, cfg):
    """Index-only host prep: per-core sqrt-weight matrices [R, V].

    Returns (masks, scale): the device computes sum(d^2 * m^2); the final
    loss is scale * sum(partials).  Weights are rescaled by the dominant
    class weight so that the vast majority of nonzero mask cells are
    exactly 1.0 — exactly representable in f16/fp8 — making the dithered
    low-precision mask essentially lossless for the dominant class.
    """
    R = cfg["rows_per_core"]
    V = cfg["n_virus"]
    n_cores = cfg["n_cores"]
    np_mdt = _mask_np_dt(cfg)
    wp = (1.0 - float(alpha)) / 2.0
    wn = float(alpha) / 2.0
    px = np.asarray(pos_x).astype(np.int64, copy=False)
    py = np.asarray(pos_y).astype(np.int64, copy=False)
    nx = np.asarray(neg_x).astype(np.int64, copy=False)
    ny = np.asarray(neg_y).astype(np.int64, copy=False)
    # dominant weight-mass class defines the scale (mask value 1.0)
    mass_p = wp * len(px)
    mass_n = wn * len(nx)
    scale = wn if mass_n >= mass_p else wp
    if scale == 0.0:
        scale = max(wp, wn, 1e-30)
    pflat = px * V + py
    nflat = nx * V + ny
    pcore = px // R
    ncore = nx // R
    shard = R * V
    masks = []
    for c in range(n_cores):
        pl = pflat[pcore == c] - c * shard
        nl = nflat[ncore == c] - c * shard
        cp = np.bincount(pl, minlength=shard)
        cn = np.bincount(nl, minlength=shard)
        w = (wp / scale) * cp.astype(np.float32) + (wn / scale) * cn.astype(
            np.float32
        )
        nz = np.flatnonzero(w)
        wv = w[nz]
        m0, malt, q = _dither_sqrt(wv, np_mdt)
        nzu = nz.astype(np.uint64)
        u = (
            ((nzu * np.uint64(2654435761)) & np.uint64(0xFFFFFFFF)) >> np.uint64(16)
        ).astype(np.float64) / 65536.0
        mv = np.where(u < q, m0, malt)
        mf = np.zeros(shard, dtype=np_mdt)
        mf[nz] = mv
        masks.append(mf.reshape(R, V))
    return masks, scale


def make_in_maps(A, B, masks, cfg):
    """Per-core input maps, applying the configured DRAM layouts."""
    R = cfg["rows_per_core"]
    pre_ab = cfg.get("pretiled", False)
    pre_m = pre_ab or cfg.get("m_pretiled", False)
    xf_ab = (lambda x: pretile(x, cfg)) if pre_ab else (lambda x: x)
    xf_m = (lambda x: pretile(x, cfg)) if pre_m else (lambda x: x)
    return [
        {
            "a": xf_ab(A[c * R: (c + 1) * R]),
            "b": xf_ab(B[c * R: (c + 1) * R]),
            "m": xf_m(masks[c]),
        }
        for c in range(cfg["n_cores"])
    ]


def run_cores(in_maps, cfg):
    global LAST_RESULTS
    from concourse.bass_utils import run_bass_kernel_spmd
    from concourse.bass_interp import get_hw_module

    key = tuple(sorted(cfg.items()))
    if key not in _BUILD_CACHE:
        _BUILD_CACHE[key] = build_nc(cfg)
    nc = _BUILD_CACHE[key]

    old_m = nc.m
    nc.m = get_hw_module(nc.m)
    try:
        res = run_bass_kernel_spmd(
            nc,
            in_maps,
            core_ids=list(range(len(in_maps))),
            trace=TRACE,
        )
    finally:
        nc.m = old_m
    LAST_RESULTS = res
    return [r["partials"] for r in res.results]


def kernel(
    drug_virus_reconstruct,
    drug_virus,
    drug_virus_mask,
    pos_x_index,
    pos_y_index,
    neg_x_index,
    neg_y_index,
    alpha,
):
    cfg = FULL_CFG
    A = np.ascontiguousarray(np.asarray(drug_virus_reconstruct, dtype=np.float32))
    B = np.ascontiguousarray(np.asarray(drug_virus, dtype=np.float32))

    masks, scale = build_masks(
        pos_x_index, pos_y_index, neg_x_index, neg_y_index, alpha, cfg
    )

    in_maps = make_in_maps(A, B, masks, cfg)

    partials = run_cores(in_maps, cfg)
    loss = scale * float(
        np.sum([np.sum(p, dtype=np.float64) for p in partials], dtype=np.float64)
    )
    return np.float32(loss)
